# revision 23
# baseline (speedup 1.0000x reference)
"""Bass/Trainium2 SPMD kernel for nn_Block3D (8 NeuronCores).

Spatial z-shard (24 planes -> 3 per core) with REDUNDANT 5-plane halo compute
(host preps a zero-padded 7-plane x slab per core), which removes the halo
AllGather.  The CAFM `mod` gate factors out of the depthwise conv and is
folded into the dynamic kernels.  kernel_net GEMV1 row-sharded / GEMV2
K-sharded with one AllReduce.  Depthwise 3x3x3 convs split between PE
(diagonalized weights built on-chip by the Pool engine) and DVE
(4x-mode tensor_scalar + 2x tensor_tensor accumulate).  LN2's affine is
folded into the project_in weights host-side.
"""

import os
from contextlib import ExitStack

import numpy as np
import ml_dtypes

import concourse.bass as bass
import concourse.bacc as bacc
import concourse.tile as tile
from concourse import mybir
from concourse.bass_utils import run_bass_kernel_spmd

BF = ml_dtypes.bfloat16
F32 = mybir.dt.float32
BF16 = mybir.dt.bfloat16

C = 768
G = 12
GD = 64
S = 24
HID = 4 * C
KK = 27
V = S * S * S
EPS = 1e-5
NCORES = 8
ZP = S // NCORES          # 3 own planes
Z7 = ZP + 4               # 7-plane x slab (halo 2)
Z5 = ZP + 2               # 5-plane compute region (halo 1)
PL = S * S                # 576
V5 = Z5 * PL              # 2880
VC = ZP * PL              # 1728
PPL = 26 * 26             # 676
PAD7 = Z7 * PPL           # 4732
PAD5 = Z5 * PPL           # 3380
CT = C // 128             # 6
HT = HID // 128           # 24
W1R = HID // NCORES       # 384
W2K = W1R
KFLAT = C * KK            # 20736
W96 = V5 // 96            # 30
HB = CT * C               # 4608

DVE_TILES = frozenset(int(x) for x in os.environ.get(
    "BLK3D_DVE_TILES", ",".join(str(i) for i in range(12, 21))).split(",")
    if x != "")
CAFM_DVE = frozenset(int(x) for x in os.environ.get(
    "BLK3D_CAFM_DVE", "4,5").split(",") if x != "")

TAPS = [(dz, dy, dx) for dz in (-1, 0, 1) for dy in (-1, 0, 1) for dx in (-1, 0, 1)]

_CACHE = {}

Copy = mybir.ActivationFunctionType.Copy
Iden = mybir.ActivationFunctionType.Identity
Gelu = mybir.ActivationFunctionType.Gelu
Sigmoid = mybir.ActivationFunctionType.Sigmoid
Square = mybir.ActivationFunctionType.Square
Sqrt = mybir.ActivationFunctionType.Sqrt
Relu = mybir.ActivationFunctionType.Relu
ADD = mybir.AluOpType.add
SUB = mybir.AluOpType.subtract
MULT = mybir.AluOpType.mult

(S_Y, S_LB1, S_LB2, S_TPB, S_ABV, S_ABO, S_OPB, S_N3W, S_N3B,
 S_GNG, S_GNB, S_MODB) = range(12)


def build_program():
    nc = bacc.Bacc("TRN2", target_bir_lowering=False)

    def dram_in(name, shape, dtype=F32):
        return nc.declare_dram_parameter(name, list(shape), dtype, isOutput=False)

    x7_in = dram_in("x7", [C, PAD7], BF16)       # padded, zero borders
    smalls = dram_in("smalls", [C, 16])
    knb1 = dram_in("knb1", [W1R])
    knb2r = dram_in("knb2r", [KFLAT])
    gind6 = dram_in("gind6", [CT, 128, G])
    gexpT = dram_in("gexpT", [G, C])
    ident = dram_in("ident", [128, 128], BF16)
    onesc = dram_in("onesc", [128, 1], BF16)
    wtext = dram_in("wtext", [128, 8 * HB], BF16)
    w2sT = dram_in("w2sT", [W2K, KFLAT], BF16)
    opT = dram_in("opT", [C, C], BF16)
    wiT = dram_in("wiT", [HT, 128, CT * 128], BF16)
    woT = dram_in("woT", [CT, 128, (HT // 2) * 128], BF16)
    dwkcat = dram_in("dwkcat", [128, HT * KK])
    biaswi = dram_in("biaswi", [128, HT * Z5])
    vmask96 = dram_in("vmask96", [96, W96])
    out = nc.declare_dram_parameter("out", [C, VC], F32, isOutput=True)

    with tile.TileContext(nc) as tc, ExitStack() as ctx:
        dram = ctx.enter_context(tc.tile_pool(name="dram", bufs=1, space="DRAM"))
        persist = ctx.enter_context(tc.tile_pool(name="persist", bufs=1))
        gpool = ctx.enter_context(tc.tile_pool(name="gemv", bufs=2))

        # ---------------- persistent small tiles ----------------
        sm = [persist.tile([128, 16], F32, name=f"sm{i}", tag=f"sm{i}")
              for i in range(CT)]
        for i in range(CT):
            nc.sync.dma_start(sm[i][:], smalls[128 * i:128 * (i + 1), :])
        id_t = persist.tile([128, 128], BF16, name="identt", tag="identt")
        nc.sync.dma_start(id_t[:], ident[:, :])
        ones_t = persist.tile([128, 1], BF16, name="onest", tag="onest")
        nc.sync.dma_start(ones_t[:], onesc[:, :])
        eps_t = persist.tile([128, 1], F32, name="epst", tag="epst")
        nc.vector.memset(eps_t[:], EPS)
        junk = persist.tile([128, VC], BF16, name="junk", tag="junk")
        dwk = persist.tile([128, HT * KK], F32, name="dwk", tag="dwk")
        nc.sync.dma_start(dwk[:], dwkcat[:, :])
        bwi = persist.tile([128, HT * Z5], F32, name="bwi", tag="bwi")
        nc.sync.dma_start(bwi[:], biaswi[:, :])
        vm96 = persist.tile([96, W96], F32, name="vm96", tag="vm96")
        nc.sync.dma_start(vm96[:], vmask96[:, :])
        knb1_t = persist.tile([128, 3], F32, name="knb1t", tag="knb1t")
        nc.sync.dma_start(
            knb1_t[:], bass.AP(tensor=knb1, offset=0, ap=[[1, 128], [128, 3]]))
        kern = [persist.tile([128, KK], F32, name=f"kern{i}", tag=f"kern{i}")
                for i in range(CT)]
        gexp_t = [persist.tile([G, 128], F32, name=f"gexp{i}", tag=f"gexp{i}")
                  for i in range(CT)]
        for i in range(CT):
            nc.sync.dma_start(gexp_t[i][:], gexpT[:, 128 * i:128 * (i + 1)])
        gind_t = [persist.tile([128, G], F32, name=f"gind{i}", tag=f"gind{i}")
                  for i in range(CT)]
        for i in range(CT):
            nc.sync.dma_start(gind_t[i][:], gind6[i, :, :])

        # big buffers: padded x slabs; cols [0:V5] of each later reused as xb
        bigp = ctx.enter_context(tc.tile_pool(name="bigp", bufs=1))
        mp7 = [bigp.tile([128, PAD7], BF16, name=f"mp{i}", tag=f"mp{i}")
               for i in range(CT)]
        xbp = ctx.enter_context(tc.tile_pool(name="xbp", bufs=1))
        xb = [xbp.tile([128, V5], BF16, name=f"xb{i}", tag=f"xb{i}")
              for i in range(CT)]

        xs = mp7
        for i in range(CT):
            q = nc.scalar if i % 2 else nc.sync
            q.dma_start(xs[i][:], x7_in[128 * i:128 * (i + 1), :])
        x4 = [xs[i].rearrange("p (z y x) -> p z y x", z=Z7, y=26, x=26)
              for i in range(CT)]

        # -------- phase A: vc sums + AR1 --------
        vcs = persist.tile([128, CT], F32, name="vcs", tag="vcs")
        for i in range(CT):
            nc.scalar.activation(junk[:], x4[i][:, 2:5, 1:25, 1:25], Copy,
                                 accum_out=vcs[:, i:i + 1])
        ar1_in = dram.tile([C], F32, name="ar1i", tag="ar1i")
        ar1_out = dram.tile([C], F32, name="ar1o", tag="ar1o",
                            addr_space="Shared")
        nc.sync.dma_start(
            bass.AP(tensor=ar1_in[:].tensor, offset=ar1_in[:].offset,
                    ap=[[1, 128], [128, CT]]), vcs[:])
        nc.gpsimd.collective_compute(
            "AllReduce", ADD, replica_groups=[list(range(NCORES))],
            ins=[ar1_in[:]], outs=[ar1_out[:]])

        # -------- text GEMV chain + kernel-net + AR2 --------
        with tc.tile_pool(name="lorap", bufs=2) as lorap, \
             tc.tile_pool(name="modwp", bufs=1) as modwp, \
             tc.tile_pool(name="kseq", bufs=3) as kseq, \
             tc.tile_pool(name="psA", bufs=2, space="PSUM") as psA:

            modw_t = modwp.tile([128, 2 * HB], BF16, name="modw", tag="modw")
            nc.sync.dma_start(modw_t[:, 0:HB], wtext[:, 5 * HB:6 * HB])
            nc.sync.dma_start(modw_t[:, HB:2 * HB], wtext[:, 6 * HB:7 * HB])
            w1s_t = modwp.tile([128, HB], BF16, name="w1s", tag="w1s")
            nc.sync.dma_start(w1s_t[:], wtext[:, 7 * HB:8 * HB])

            def gemv(wtile, in_cols, nk, nm, act, bias_cols, tag, scale=1.0,
                     odt=BF16):
                ps = psA.tile([128, nm], F32, name="ps_g", tag="ps_g")
                for m in range(nm):
                    for k in range(nk):
                        nc.tensor.matmul(
                            ps[:, m:m + 1], wtile[:, C * k + 128 * m:
                                                  C * k + 128 * m + 128],
                            in_cols[k][:], start=(k == 0), stop=(k == nk - 1))
                bt = gpool.tile([128, nm], F32, name=f"{tag}b", tag="gemvb")
                for m in range(nm):
                    if bias_cols is not None:
                        nc.vector.tensor_scalar(bt[:, m:m + 1], ps[:, m:m + 1],
                                                bias_cols[m], None, op0=ADD)
                    else:
                        nc.vector.tensor_scalar(bt[:, m:m + 1], ps[:, m:m + 1],
                                                0.0, None, op0=ADD)
                ob = gpool.tile([128, nm], odt, name=f"{tag}o", tag=f"{tag}o")
                nc.scalar.activation(ob[:], bt[:], act, scale=scale)
                return [ob[:, m:m + 1] for m in range(nm)]

            def lora_w(stage):
                t = lorap.tile([128, HB], BF16, name="lw", tag="lw")
                nc.sync.dma_start(t[:], wtext[:, HB * stage:HB * (stage + 1)])
                return t

            t_cols = []
            for i in range(CT):
                t = gpool.tile([128, 1], BF16, name=f"tc{i}", tag=f"tc{i}")
                nc.scalar.activation(t[:], sm[i][:, S_Y:S_Y + 1], Copy)
                t_cols.append(t)
            h1 = gemv(lora_w(0), t_cols, CT, CT, Relu,
                      [sm[i][:, S_LB1:S_LB1 + 1] for i in range(CT)], "lw1")
            h2 = gemv(lora_w(1), h1, CT, CT, Iden,
                      [sm[i][:, S_LB2:S_LB2 + 1] for i in range(CT)], "lw2")
            tp = gemv(lora_w(2), h2, CT, CT, Iden,
                      [sm[i][:, S_TPB:S_TPB + 1] for i in range(CT)], "tpw")
            av = gemv(lora_w(3), tp, CT, CT, Iden,
                      [sm[i][:, S_ABV:S_ABV + 1] for i in range(CT)], "avw")
            attn = gemv(lora_w(4), av, CT, CT, Iden,
                        [sm[i][:, S_ABO:S_ABO + 1] for i in range(CT)], "aow")

            comb = []
            for i in range(CT):
                cb = gpool.tile([128, 1], BF16, name=f"cmb{i}", tag=f"cmb{i}")
                col = persist.tile([128, 1], F32, name=f"vcc{i}",
                                   tag=f"vcc{i}")
                nc.sync.dma_start(
                    col[:], bass.AP(tensor=ar1_out[:].tensor,
                                    offset=ar1_out[:].offset + 128 * i,
                                    ap=[[1, 128], [128, 1]]))
                nc.scalar.activation(cb[:], col[:], Copy, scale=1.0 / V)
                comb.append(cb)
            comb += attn

            mod = gemv(modw_t, comb, 2 * CT, CT, Sigmoid,
                       [sm[i][:, S_MODB:S_MODB + 1] for i in range(CT)],
                       "modw", odt=F32)

            kp1 = []
            for m in range(3):
                ps = psA.tile([128, 1], F32, name="ps_small", tag="ps_small")
                for k in range(2 * CT):
                    nc.tensor.matmul(
                        ps[:], w1s_t[:, W1R * k + 128 * m:
                                     W1R * k + 128 * m + 128],
                        comb[k][:], start=(k == 0), stop=(k == 2 * CT - 1))
                o = gpool.tile([128, 1], BF16, name=f"kp1o{m}", tag=f"kp1o{m}")
                nc.scalar.activation(o[:], ps[:], Relu,
                                     bias=knb1_t[:, m:m + 1])
                kp1.append(o)

            ar2_in = dram.tile([KFLAT], F32, name="ar2i", tag="ar2i")
            ar2_out = dram.tile([KFLAT], F32, name="ar2o", tag="ar2o",
                                addr_space="Shared")
            TCH = [(0, 4), (4, 4), (8, 4), (12, 4), (16, 4), (20, 4), (24, 3)]
            for c0, cn in TCH:
                wts = []
                for k in range(3):
                    wtile = kseq.tile([128, 4 * C], BF16, name=f"w2c{k}",
                                      tag=f"w2c{k}")
                    nc.sync.dma_start(
                        wtile[:, 0:cn * C],
                        w2sT[128 * k:128 * (k + 1), C * c0:C * (c0 + cn)])
                    wts.append(wtile)
                for tt in range(cn):
                    t = c0 + tt
                    ps = psA.tile([128, CT], F32, name="g2ps", tag="g2ps")
                    for m in range(CT):
                        for k in range(3):
                            nc.tensor.matmul(
                                ps[:, m:m + 1],
                                wts[k][:, C * tt + 128 * m:
                                       C * tt + 128 * m + 128],
                                kp1[k][:], start=(m == 0 and k == 0),
                                stop=(m == CT - 1 and k == 2))
                    ko = kseq.tile([128, CT], F32, name="g2o", tag="g2o")
                    nc.scalar.activation(ko[:], ps[:], Copy)
                    nc.sync.dma_start(
                        bass.AP(tensor=ar2_in[:].tensor,
                                offset=ar2_in[:].offset + C * t,
                                ap=[[1, 128], [128, CT]]),
                        ko[:])
            nc.gpsimd.collective_compute(
                "AllReduce", ADD, replica_groups=[list(range(NCORES))],
                ins=[ar2_in[:]], outs=[ar2_out[:]])

            # kern = mod * (AR2 + bias): mod factors out of the depthwise conv
            for i in range(CT):
                kb = kseq.tile([128, KK], F32, name="kbld", tag="kbld")
                nc.sync.dma_start(
                    kb[:], bass.AP(tensor=knb2r, offset=128 * i,
                                   ap=[[1, 128], [C, KK]]))
                kt0 = kseq.tile([128, KK], F32, name="kbld2", tag="kbld2")
                nc.sync.dma_start(
                    kt0[:], bass.AP(tensor=ar2_out[:].tensor,
                                    offset=ar2_out[:].offset + 128 * i,
                                    ap=[[1, 128], [C, KK]]))
                nc.vector.tensor_add(kern[i][:], kt0[:], kb[:])
                nc.vector.tensor_scalar_mul(kern[i][:], kern[i][:],
                                            mod[i][:, 0:1])

        # -------- phase C: dynamic conv + GN --------
        cafm_pe = [i for i in range(CT) if i not in CAFM_DVE]
        with tc.tile_pool(name="dynp", bufs=1) as dynp, \
             tc.tile_pool(name="cvtmp", bufs=1) as cvtmp:
            dyn = [dynp.tile([128, V5], BF16, name=f"dyn{i}", tag=f"dyn{i}")
                   for i in range(CT)]
            with tc.tile_pool(name="dgdyn", bufs=1) as dgdyn_pool, \
                 tc.tile_pool(name="cvps", bufs=3, space="PSUM") as cvps, \
                 tc.tile_pool(name="gnps", bufs=1, space="PSUM") as gnps, \
                 tc.tile_pool(name="gnst", bufs=2) as gnstp:

                def conv_pe_plane(dst, p4, dga, zo):
                    for hb in range(2):
                        ps = cvps.tile([128, PL // 2], F32, name="cv",
                                       tag="cv")
                        for ti, (dz, dy, dx) in enumerate(TAPS):
                            sv = p4[:, zo + dz,
                                    1 + dy + 12 * hb:13 + dy + 12 * hb,
                                    1 + dx:25 + dx]
                            nc.tensor.matmul(
                                ps[:], dga[:, 128 * ti:128 * (ti + 1)], sv,
                                start=(ti == 0), stop=(ti == KK - 1))
                        nc.scalar.activation(
                            dst[:, (zo - 1) * PL + 288 * hb:
                                (zo - 1) * PL + 288 * (hb + 1)],
                            ps[:], Copy)

                def conv_dve(dst, mp, p4, ktile, planes):
                    """per-plane f32-accumulated stt chains -> dyn bf16."""
                    for zo in planes:
                        acc = cvtmp.tile([128, PL], F32, name="cvt", tag="cvt")
                        for ti, (dz, dy, dx) in enumerate(TAPS):
                            sv = p4[:, zo + dz, 1 + dy:25 + dy, 1 + dx:25 + dx]
                            if ti == 0:
                                nc.vector.tensor_scalar_mul(acc[:], sv,
                                                            ktile[:, 0:1])
                            else:
                                nc.vector.scalar_tensor_tensor(
                                    acc[:], sv, ktile[:, ti:ti + 1], acc[:],
                                    op0=MULT, op1=ADD)
                        nc.scalar.activation(dst[:, (zo - 1) * PL:zo * PL],
                                             acc[:], Copy)

                dga_dyn = {}
                for i in range(CT):
                    dga = dgdyn_pool.tile([128, KK * 128], BF16, name="dgd",
                                          tag=f"dgd{i}")
                    for ti in range(KK):
                        nc.gpsimd.tensor_scalar_mul(
                            dga[:, 128 * ti:128 * (ti + 1)], id_t[:],
                            kern[i][:, ti:ti + 1])
                    dga_dyn[i] = dga

                gps = gnps.tile([G, 2], F32, name="gps", tag="gps")
                for i in cafm_pe:
                    for zo in (2, 3, 4):
                        conv_pe_plane(dyn[i], x4[i], dga_dyn[i], zo)
                for i in sorted(CAFM_DVE):
                    conv_dve(dyn[i], xs[i], x4[i], kern[i], (2, 3, 4))
                for i in range(CT):
                    st = gnstp.tile([128, 2], F32, name="gnst", tag="gnst")
                    nc.scalar.activation(junk[:], dyn[i][:, PL:4 * PL], Copy,
                                         accum_out=st[:, 0:1])
                    nc.scalar.activation(junk[:], dyn[i][:, PL:4 * PL],
                                         Square, accum_out=st[:, 1:2])
                    nc.tensor.matmul(gps[:], gind_t[i][:], st[:],
                                     start=(i == 0), stop=(i == CT - 1))
                gsb = persist.tile([G, 2], F32, name="gsb", tag="gsb")
                nc.scalar.activation(gsb[:], gps[:], Copy)
                ar3_in = dram.tile([G, 2], F32, name="ar3i", tag="ar3i")
                ar3_out = dram.tile([G, 2], F32, name="ar3o", tag="ar3o",
                                    addr_space="Shared")
                nc.sync.dma_start(ar3_in[:], gsb[:])
                nc.gpsimd.collective_compute(
                    "AllReduce", ADD, replica_groups=[list(range(NCORES))],
                    ins=[ar3_in[:]], outs=[ar3_out[:]])

                for i in range(CT):
                    if i == CT - 1:
                        conv_dve(dyn[i], xs[i], x4[i], kern[i], (1, 5))
                    else:
                        for zo in (1, 5):
                            conv_pe_plane(dyn[i], x4[i], dga_dyn[i], zo)

                gstat = persist.tile([G, 2], F32, name="gstat", tag="gstat")
                nc.sync.dma_start(gstat[:], ar3_out[:])
                NGRP = float(GD * V)
                gmr = persist.tile([G, 2], F32, name="gmr", tag="gmr")
                nc.scalar.activation(gmr[:, 0:1], gstat[:, 0:1], Copy,
                                     scale=1.0 / NGRP)
                musq = persist.tile([G, 1], F32, name="musq", tag="musq")
                nc.scalar.square(musq[:], gmr[:, 0:1])
                var = persist.tile([G, 1], F32, name="gvar", tag="gvar")
                nc.vector.tensor_scalar(var[:], gstat[:, 1:2], 1.0 / NGRP,
                                        None, op0=MULT)
                nc.vector.tensor_sub(var[:], var[:], musq[:])
                nc.scalar.activation(var[:], var[:], Sqrt,
                                     bias=eps_t[0:G, 0:1])
                nc.vector.reciprocal(gmr[:, 1:2], var[:])

            # -------- cafm matmul + xb = x * cafm; sq = xb^2 --------
            with tc.tile_pool(name="opw", bufs=1) as opw_pool, \
                 tc.tile_pool(name="psB", bufs=1, space="PSUM") as psB, \
                 tc.tile_pool(name="opps", bufs=3, space="PSUM") as opps:
                opT_t = [opw_pool.tile([128, C], BF16, name=f"opT{i}",
                                       tag=f"opT{i}") for i in range(CT)]
                for i in range(CT):
                    nc.scalar.dma_start(opT_t[i][:],
                                        opT[128 * i:128 * (i + 1), :])
                cafm_shift = []
                gsc = []
                for i in range(CT):
                    ps = psB.tile([128, 2], F32, name="ps_sm2", tag="ps_sm2")
                    nc.tensor.matmul(ps[:], gexp_t[i][:], gmr[:], start=True,
                                     stop=True)
                    mu_c = persist.tile([128, 2], F32, name=f"muc{i}",
                                        tag=f"muc{i}")
                    nc.scalar.activation(mu_c[:], ps[:], Copy)
                    a = persist.tile([128, 1], F32, name=f"gsc{i}",
                                     tag=f"gsc{i}")
                    nc.vector.tensor_mul(a[:], sm[i][:, S_GNG:S_GNG + 1],
                                         mu_c[:, 1:2])
                    b = persist.tile([128, 1], F32, name=f"gsh{i}",
                                     tag=f"gsh{i}")
                    nc.vector.tensor_mul(b[:], mu_c[:, 0:1], a[:])
                    nc.vector.tensor_sub(b[:], sm[i][:, S_GNB:S_GNB + 1],
                                         b[:])
                    gsc.append(a)
                    bb = gpool.tile([128, 1], BF16, name=f"gshb{i}",
                                    tag=f"gshb{i}")
                    nc.scalar.activation(bb[:], b[:], Copy)
                    cafm_shift.append(bb)
                cb_cols = []
                for m in range(CT):
                    ps = psB.tile([128, 1], F32, name="ps_sm1", tag="ps_sm1")
                    for k in range(CT):
                        nc.tensor.matmul(ps[:],
                                         opT_t[k][:, 128 * m:128 * (m + 1)],
                                         cafm_shift[k][:], start=(k == 0),
                                         stop=(k == CT - 1))
                    o = persist.tile([128, 1], F32, name=f"cbc{m}",
                                     tag=f"cbc{m}")
                    nc.scalar.activation(o[:], ps[:], Iden,
                                         bias=sm[m][:, S_OPB:S_OPB + 1])
                    cb_cols.append(o)
                for i in range(CT):
                    nc.vector.tensor_scalar_mul(opT_t[i][:], opT_t[i][:],
                                                gsc[i][:])

                NBC = 480
                for m in range(CT):
                    for b in range(V5 // NBC):
                        ps = opps.tile([128, NBC], F32, name="opw_ps",
                                       tag="opw_ps")
                        for k in range(CT):
                            nc.tensor.matmul(
                                ps[:], opT_t[k][:, 128 * m:128 * (m + 1)],
                                dyn[k][:, NBC * b:NBC * (b + 1)],
                                start=(k == 0), stop=(k == CT - 1))
                        c0 = NBC * b
                        left = NBC
                        src0 = 0
                        while left > 0:
                            z = c0 // PL
                            zoff = c0 - z * PL
                            take = min(left, PL - zoff)
                            y0 = zoff // S
                            nrows = take // S
                            nc.vector.scalar_tensor_tensor(
                                xb[m][:, c0:c0 + take],
                                ps[:, src0:src0 + take], cb_cols[m][:],
                                x4[m][:, 1 + z, 1 + y0:1 + y0 + nrows, 1:25],
                                op0=ADD, op1=MULT)
                            c0 += take
                            src0 += take
                            left -= take

        # ---------------- LN stats helper ----------------
        def ln_stats(tiles, col0, ncols, mask, tag, pool, sq_dve=False):
            w = ncols // 96
            with tc.tile_pool(name=f"{tag}ps", bufs=2, space="PSUM") as lps, \
                 tc.tile_pool(name=f"{tag}sq", bufs=2) as sqpool, \
                 tc.tile_pool(name=f"{tag}rw", bufs=1) as rwp:
                row = rwp.tile([1, 2 * ncols], F32, name="row", tag="row")
                CH = 480 if ncols % 480 == 0 else 432
                for ch0 in range(0, ncols, CH):
                    ps1 = lps.tile([1, CH], F32, name="s1", tag="s1")
                    ps2 = lps.tile([1, CH], F32, name="s2", tag="s2")
                    for k in range(CT):
                        nc.tensor.matmul(
                            ps1[:], ones_t[:],
                            tiles[k][:, col0 + ch0:col0 + ch0 + CH],
                            start=(k == 0), stop=(k == CT - 1))
                    for k in range(CT):
                        sq = sqpool.tile([128, CH], BF16, name="sq", tag="sq")
                        tk = tiles[k][:, col0 + ch0:col0 + ch0 + CH]
                        if sq_dve:
                            nc.vector.tensor_mul(sq[:], tk, tk)
                        else:
                            nc.scalar.activation(sq[:], tk, Square)
                        nc.tensor.matmul(
                            ps2[:], ones_t[:], sq[:],
                            start=(k == 0), stop=(k == CT - 1))
                    nc.scalar.activation(row[:, ch0:ch0 + CH], ps1[:], Copy,
                                         scale=1.0 / C)
                    nc.scalar.activation(row[:, ncols + ch0:ncols + ch0 + CH],
                                         ps2[:], Copy, scale=1.0 / C)
                drow = dram.tile([2 * ncols], F32, name=f"{tag}dr",
                                 tag=f"{tag}dr")
                nc.sync.dma_start(drow[:], row[:])
                rs = rwp.tile([96, 2 * w], F32, name="rs", tag="rs")
                nc.sync.dma_start(
                    rs[:], bass.AP(tensor=drow[:].tensor,
                                   offset=drow[:].offset,
                                   ap=[[w, 96], [ncols, 2], [1, w]]))
                m2 = rwp.tile([96, w], F32, name="m2", tag="m2")
                nc.scalar.square(m2[:], rs[:, 0:w])
                vr = rwp.tile([96, w], F32, name="vr", tag="vr")
                nc.vector.tensor_sub(vr[:], rs[:, w:2 * w], m2[:])
                nc.scalar.activation(vr[:], vr[:], Sqrt, bias=eps_t[0:96, 0:1])
                nc.vector.reciprocal(vr[:], vr[:])
                if mask is not None:
                    nc.vector.tensor_mul(vr[:], vr[:], mask[:, 0:w])
                drow2 = dram.tile([2 * ncols], BF16, name=f"{tag}d2",
                                  tag=f"{tag}d2")
                nc.gpsimd.dma_start(
                    bass.AP(tensor=drow2[:].tensor, offset=drow2[:].offset,
                            ap=[[w, 96], [1, w]]), rs[:, 0:w])
                nc.gpsimd.dma_start(
                    bass.AP(tensor=drow2[:].tensor,
                            offset=drow2[:].offset + ncols,
                            ap=[[w, 96], [1, w]]), vr[:])
            muB = pool.tile([128, ncols], BF16, name=f"{tag}muB",
                            tag=f"{tag}muB")
            rsB = pool.tile([128, ncols], BF16, name=f"{tag}rsB",
                            tag=f"{tag}rsB")
            nc.sync.dma_start(
                muB[:], bass.AP(tensor=drow2[:].tensor,
                                offset=drow2[:].offset,
                                ap=[[0, 128], [1, ncols]]))
            nc.sync.dma_start(
                rsB[:], bass.AP(tensor=drow2[:].tensor,
                                offset=drow2[:].offset + ncols,
                                ap=[[0, 128], [1, ncols]]))
            return muB, rsB

        # ---------------- LN2 + MLP + Wo ----------------
        with tc.tile_pool(name="xlnp", bufs=1) as xlnp, \
             tc.tile_pool(name="ln2bp", bufs=1) as ln2bp:
            muB, rsB = ln_stats(xb, 0, V5, vm96, "ln2", ln2bp, sq_dve=True)
            xln = [xlnp.tile([128, V5], BF16, name=f"xln{i}", tag=f"xln{i}")
                   for i in range(CT)]
            for i in range(CT):
                nc.vector.tensor_sub(xln[i][:], xb[i][:], muB[:])
                nc.vector.tensor_mul(xln[i][:], xln[i][:], rsB[:])

            gate = [mp7[j // 2][:, VC * (j % 2):VC * (j % 2 + 1)]
                    for j in range(HT // 2)]

            wo_walls = []
            for m in range(CT):
                wall = xlnp.tile([128, (HT // 2) * 128], BF16, name="woall",
                                 tag=f"woall{m % 3}")
                nc.sync.dma_start(wall[:], woT[m, :, :])
                wo_walls.append(wall)
            with tc.tile_pool(name="hpadp", bufs=1) as hpad_pool, \
                 tc.tile_pool(name="wiw", bufs=1) as wiw, \
                 tc.tile_pool(name="diag", bufs=1) as dpool, \
                 tc.tile_pool(name="conv1p", bufs=1) as conv1p, \
                 tc.tile_pool(name="wips", bufs=4, space="PSUM") as wips, \
                 tc.tile_pool(name="cvps2", bufs=4, space="PSUM") as cvps2:

                NBW = 480
                _memset_done = set()

                def wi_interior_copy(hp4, ps, b, tt):
                    c0 = NBW * b
                    left = NBW
                    src0 = 0
                    while left > 0:
                        z = c0 // PL
                        zoff = c0 - z * PL
                        take = min(left, PL - zoff)
                        y0 = zoff // S
                        nrows = take // S
                        bias = bwi[:, Z5 * tt + z:Z5 * tt + z + 1]
                        nc.scalar.activation(
                            hp4[:, z, 1 + y0:1 + y0 + nrows, 1:25],
                            ps[:, src0:src0 + take], Iden, bias=bias)
                        c0 += take
                        src0 += take
                        left -= take

                def mlp_tile(tt, on_dve, cidx, htag):
                    wall = wiw.tile([128, CT * 128], BF16, name="wiall",
                                    tag=f"wiall{cidx}")
                    nc.scalar.dma_start(wall[:], wiT[tt, :, :])
                    hp = hpad_pool.tile([128, PAD5], BF16, name="hpad",
                                        tag=f"hpad{htag}")
                    if htag not in _memset_done:
                        _memset_done.add(htag)
                        nc.gpsimd.memset(hp[:], 0.0)
                    hp4 = hp.rearrange("p (z y x) -> p z y x", z=Z5, y=26,
                                       x=26)
                    for b in range(V5 // NBW):
                        ps = wips.tile([128, NBW], F32, name="wi_ps",
                                       tag="wi_ps")
                        for k in range(CT):
                            nc.tensor.matmul(
                                ps[:], wall[:, 128 * k:128 * (k + 1)],
                                xln[k][:, NBW * b:NBW * (b + 1)],
                                start=(k == 0), stop=(k == CT - 1))
                        wi_interior_copy(hp4, ps, b, tt)
                    if on_dve:
                        cdst = conv1p.tile([128, VC], F32, name="cdve",
                                           tag=f"cdve{cidx}")
                        for zo in (1, 2, 3):
                            dv = cdst[:, (zo - 1) * PL:zo * PL]
                            for ti, (dz, dy, dx) in enumerate(TAPS):
                                sv = hp4[:, zo + dz, 1 + dy:25 + dy,
                                         1 + dx:25 + dx]
                                kcol = dwk[:, KK * tt + ti:KK * tt + ti + 1]
                                if ti == 0:
                                    nc.vector.tensor_scalar_mul(dv, sv, kcol)
                                else:
                                    nc.vector.scalar_tensor_tensor(
                                        dv, sv, kcol, dv, op0=MULT, op1=ADD)
                        return None, cdst
                    dga = dpool.tile([128, KK * 128], BF16, name="dg",
                                     tag=f"dg{cidx}")
                    for ti in range(KK):
                        nc.gpsimd.tensor_scalar_mul(
                            dga[:, 128 * ti:128 * (ti + 1)], id_t[:],
                            dwk[:, KK * tt + ti:KK * tt + ti + 1])
                    cps = []
                    for zo in (1, 2, 3):
                        for hb in range(2):
                            ps = cvps2.tile([128, PL // 2], F32, name="cvm",
                                            tag="cvm")
                            for ti, (dz, dy, dx) in enumerate(TAPS):
                                sv = hp4[:, zo + dz,
                                         1 + dy + 12 * hb:13 + dy + 12 * hb,
                                         1 + dx:25 + dx]
                                nc.tensor.matmul(
                                    ps[:], dga[:, 128 * ti:128 * (ti + 1)], sv,
                                    start=(ti == 0), stop=(ti == KK - 1))
                            cps.append(ps)
                    return cps, None

                PAIR_ORDER = [0, 1, 2, 9, 3, 4, 10, 5, 6, 11, 7, 8]
                for j in PAIR_ORDER:
                    both_pe = (j + HT // 2) not in DVE_TILES
                    p1, s1 = mlp_tile(j, j in DVE_TILES, 0, 0)
                    g1 = conv1p.tile([128, VC], BF16, name="gelu1",
                                     tag="gelu1")
                    if p1 is not None:
                        for z in range(6):
                            nc.scalar.activation(
                                g1[:, 288 * z:288 * (z + 1)], p1[z][:], Gelu)
                    else:
                        nc.scalar.activation(g1[:], s1[:], Gelu)
                    p2, s2 = mlp_tile(j + HT // 2,
                                      (j + HT // 2) in DVE_TILES, 1,
                                      0 if both_pe else 1 + (j % 2))
                    if p2 is not None:
                        for z in range(6):
                            nc.vector.tensor_mul(
                                gate[j][:, 288 * z:288 * (z + 1)],
                                g1[:, 288 * z:288 * (z + 1)], p2[z][:])
                    else:
                        nc.vector.tensor_mul(gate[j][:], g1[:], s2[:])

            with tc.tile_pool(name="wops", bufs=3, space="PSUM") as wops:
                NBO = 432
                for m in range(CT):
                    wall = wo_walls[m]
                    for b in range(VC // NBO):
                        ps = wops.tile([128, NBO], F32, name="wo_ps",
                                       tag="wo_ps")
                        for k in range(HT // 2):
                            nc.tensor.matmul(
                                ps[:], wall[:, 128 * k:128 * (k + 1)],
                                gate[k][:, NBO * b:NBO * (b + 1)],
                                start=(k == 0), stop=False)
                        nc.tensor.matmul(
                            ps[:], id_t[:],
                            xb[m][:, PL + NBO * b:PL + NBO * (b + 1)],
                            start=False, stop=True)
                        nc.scalar.activation(
                            xb[m][:, PL + NBO * b:PL + NBO * (b + 1)], ps[:],
                            Copy)

        # ---------------- LN3 + output ----------------
        with tc.tile_pool(name="ln3bp", bufs=1) as ln3bp:
            muB3, rsB3 = ln_stats(xb, PL, VC, None, "ln3", ln3bp, sq_dve=True)
            with tc.tile_pool(name="glueH", bufs=2) as glueH:
                for i in range(CT):
                    t1 = glueH.tile([128, VC], BF16, name="ln3t", tag="ln3t")
                    nc.vector.tensor_sub(t1[:], xb[i][:, PL:4 * PL], muB3[:])
                    of = glueH.tile([128, VC], F32, name="outf", tag="outf")
                    nc.vector.scalar_tensor_tensor(
                        of[:], t1[:], sm[i][:, S_N3W:S_N3W + 1], rsB3[:],
                        op0=MULT, op1=MULT)
                    nc.vector.tensor_scalar(of[:], of[:],
                                            sm[i][:, S_N3B:S_N3B + 1], None,
                                            op0=ADD)
                    nc.scalar.dma_start(out[128 * i:128 * (i + 1), :], of[:])

    nc.compile()
    return nc


def _prep(inputs):
    bf = lambda a: np.ascontiguousarray(a).astype(BF)
    f32 = lambda a: np.ascontiguousarray(a, dtype=np.float32)
    x = f32(inputs["x"][0])
    xf = x.reshape(C, S, S, S)

    smalls = np.zeros((C, 16), np.float32)
    smalls[:, 0] = f32(inputs["y"][0, 0])
    for i, k in enumerate(["lora_b1", "lora_b2", "tp_b", "attn_bv", "attn_bo",
                           "op_b", "n3_w", "n3_b", "gn_g", "gn_b", "mod_b"]):
        smalls[:, i + 1] = f32(inputs[k])

    gind6 = np.zeros((CT, 128, G), np.float32)
    for j in range(CT):
        for p in range(128):
            gind6[j, p, (128 * j + p) // GD] = 1.0
    gexpT = np.zeros((G, C), np.float32)
    for c in range(C):
        gexpT[c // GD, c] = 1.0

    kn_W2 = f32(inputs["kn_W2"])
    w2r = kn_W2.reshape(C, KK, HID).transpose(1, 0, 2).reshape(KFLAT, HID)
    w2T = np.ascontiguousarray(w2r.T)
    knb2r = f32(inputs["kn_b2"]).reshape(C, KK).T.copy().reshape(-1)
    kn_W1 = f32(inputs["kn_W1"])

    def wcat_blocks(wT):
        kt = wT.shape[0] // 128
        return wT.reshape(kt, 128, wT.shape[1]).transpose(1, 0, 2).reshape(
            128, kt * wT.shape[1])

    wtext_com = np.concatenate([
        wcat_blocks(f32(inputs["lora_W1"]).T),
        wcat_blocks(f32(inputs["lora_W2"]).T),
        wcat_blocks(f32(inputs["tp_W"]).T),
        wcat_blocks(f32(inputs["attn_Wv"]).T),
        wcat_blocks(f32(inputs["attn_Wo"]).T),
        wcat_blocks(f32(inputs["mod_W"]).T),
    ], axis=1)

    n2w = f32(inputs["n2_w"]); n2b = f32(inputs["n2_b"])
    Wi = f32(inputs["mlp_Wi"]) * n2w[None, :]
    bvec = Wi @ n2b
    WiT = np.ascontiguousarray(Wi.T)
    wiT = np.zeros((HT, 128, CT * 128), np.float32)
    for tt in range(HT):
        for j in range(CT):
            wiT[tt, :, 128 * j:128 * (j + 1)] = WiT[128 * j:128 * (j + 1),
                                                    128 * tt:128 * (tt + 1)]
    WoT = np.ascontiguousarray(f32(inputs["mlp_Wo"]).T)
    woT = np.zeros((CT, 128, (HT // 2) * 128), np.float32)
    for m in range(CT):
        for k in range(HT // 2):
            woT[m, :, 128 * k:128 * (k + 1)] = WoT[128 * k:128 * (k + 1),
                                                   128 * m:128 * (m + 1)]

    mlp_dw = f32(inputs["mlp_dw"]).reshape(HID, KK)
    dwkcat = mlp_dw.reshape(HT, 128, KK).transpose(1, 0, 2).reshape(
        128, HT * KK)

    com = dict(
        smalls=smalls, knb2r=knb2r,
        gind6=gind6, gexpT=gexpT,
        ident=bf(np.eye(128, dtype=np.float32)),
        onesc=bf(np.ones((128, 1), np.float32)),
        opT=bf(f32(inputs["op_W"]).T),
        wiT=bf(wiT), woT=bf(woT),
        dwkcat=dwkcat,
    )

    in_maps = []
    for i in range(NCORES):
        z0 = ZP * i
        # padded 7-plane slab: global z0-2 .. z0+4, 26x26 planes, zero borders
        xh = np.zeros((C, Z7, 26, 26), np.float32)
        lo, hi = max(z0 - 2, 0), min(z0 + ZP + 2, S)
        xh[:, lo - (z0 - 2):lo - (z0 - 2) + (hi - lo), 1:25, 1:25] = \
            xf[:, lo:hi]
        valid = np.array([0 <= z0 - 1 + z < S for z in range(Z5)], np.float32)
        vmask = np.zeros((96, W96), np.float32)
        for r in range(96):
            for cc in range(W96):
                vmask[r, cc] = valid[(r * W96 + cc) // PL]
        bw = np.zeros((128, HT * Z5), np.float32)
        for tt in range(HT):
            for z in range(Z5):
                bw[:, Z5 * tt + z] = bvec[128 * tt:128 * (tt + 1)] * valid[z]
        wtext = np.concatenate([
            wtext_com,
            wcat_blocks(kn_W1[W1R * i:W1R * (i + 1), :].T),
        ], axis=1)
        m = dict(com)
        m.update(
            x7=xh.reshape(C, PAD7).astype(BF),
            knb1=f32(inputs["kn_b1"][W1R * i:W1R * (i + 1)]),
            wtext=bf(wtext),
            w2sT=bf(w2T[W2K * i:W2K * (i + 1), :]),
            biaswi=bw,
            vmask96=vmask,
        )
        in_maps.append(m)
    return in_maps


def kernel(**inputs) -> np.ndarray:
    if "nc" not in _CACHE:
        _CACHE["nc"] = build_program()
    nc = _CACHE["nc"]
    in_maps = _prep(inputs)
    res = run_bass_kernel_spmd(nc, in_maps, list(range(NCORES)))
    outs = [res.results[i]["out"].reshape(C, ZP, PL) for i in range(NCORES)]
    full = np.concatenate(outs, axis=1)
    return full.reshape(1, C, S, S, S).astype(np.float32)
